# revision 1
# baseline (speedup 1.0000x reference)
"""Trainium2 Bass kernel for nn_Attention (B=2,T=8,N=512,C=768,H=12).

Strategy: data-parallel over the 16 (b,t) slices -> 2 slices per core, 8 cores.
All weight/mask transposes are done on host. On-chip per slice:
  xT = transpose(x)                      (PE transposes, 128x128 tiles)
  qkvT[q,k] = W_qk @ xT                  ([d, n] layout; scale folded into Wq on host)
  v = xT.T @ W_v                         ([token, d] layout)
  ST[m,n] = kT.T @ qT ; P = exp(ST + maskT)   (no max-subtraction: logits bounded)
  OT'[0:64] = v_h.T @ P (PV), OT'[64] = row-sums l (ones column in v tiles)
  outT[c,n] = OT' * broadcast(1/l)       (ones-matmul broadcast of recip row)
  y = outT.T @ proj_wT                   ([n, C] layout, DMA out)
All matmuls run in float32r (full PE rate at N>=256).
"""
import sys

sys.path.insert(0, "/opt/trn_rl_repo")

import numpy as np
import concourse.bacc as bacc
import concourse.mybir as mybir
import concourse.tile as tile
from concourse.bass_utils import run_bass_kernel_spmd
from concourse.masks import make_identity

B, T, N, C = 2, 8, 512, 768
H = 12
Dh = C // H            # 64
SL = 2                 # slices per core
NCORES = 8
NC4 = N // 128         # 4 n-chunks
CC6 = C // 128         # 6 c-chunks
F32 = mybir.dt.float32
F32R = mybir.dt.float32r

_cache = {}


def build_nc():
    nc = bacc.Bacc()
    xs = nc.dram_tensor("xs", [SL, N, C], F32R, kind="ExternalInput")
    qkv_wTqk = nc.dram_tensor("qkv_wTqk", [C, 2 * C], F32R, kind="ExternalInput")
    qkv_wTv = nc.dram_tensor("qkv_wTv", [C, C], F32R, kind="ExternalInput")
    proj_wT = nc.dram_tensor("proj_wT", [C, C], F32R, kind="ExternalInput")
    maskT = nc.dram_tensor("maskT", [N, N], F32R, kind="ExternalInput")
    y = nc.dram_tensor("y", [SL, N, C], F32, kind="ExternalOutput")

    with tile.TileContext(nc) as tc:
        with (
            tc.tile_pool(name="wpool", bufs=1) as wpool,
            tc.tile_pool(name="sb", bufs=1) as sb,
            tc.tile_pool(name="ps", bufs=1, space="PSUM") as ps,
        ):
            # ---- persistent weights ----
            qkw = [wpool.tile([128, 2 * C], F32R, tag=f"qkw{cc}", name=f"qkw{cc}") for cc in range(CC6)]
            vw = [wpool.tile([128, C], F32R, tag=f"vw{cc}", name=f"vw{cc}") for cc in range(CC6)]
            projw = [wpool.tile([128, C], F32R, tag=f"projw{cc}", name=f"projw{cc}") for cc in range(CC6)]
            maskt = [wpool.tile([128, N], F32R, tag=f"maskt{mc}", name=f"maskt{mc}") for mc in range(NC4)]
            def emit_weight_dmas():
                for cc in range(CC6):
                    eng = nc.gpsimd if cc % 2 == 0 else nc.scalar
                    eng.dma_start(vw[cc][:], qkv_wTv[128 * cc:128 * (cc + 1), :])
                for cc in range(CC6):
                    eng = (nc.gpsimd, nc.scalar)[cc % 2]
                    eng.dma_start(qkw[cc][:], qkv_wTqk[128 * cc:128 * (cc + 1), :])
                for mc in range(NC4):
                    nc.sync.dma_start(maskt[mc][:], maskT[128 * mc:128 * (mc + 1), :])

            def emit_projw_dmas():
                for cc in range(CC6):
                    nc.gpsimd.dma_start(projw[cc][:], proj_wT[128 * cc:128 * (cc + 1), :])
            identf = wpool.tile([128, 128], F32, tag="identf")
            make_identity(nc, identf[:])
            ident = wpool.tile([128, 128], F32R, tag="ident")
            nc.vector.tensor_copy(ident[:], identf[:])
            onesf = wpool.tile([128, Dh], F32, tag="onesf")
            nc.gpsimd.memset(onesf[:], 1.0)

            xTs = [[None] * CC6 for _ in range(SL)]
            vsbs = [[None] * NC4 for _ in range(SL)]
            qks = [[None] * (2 * CC6) for _ in range(SL)]
            outTs = [[None] * CC6 for _ in range(SL)]

            def get(lst, i, mk):
                if lst[i] is None:
                    lst[i] = mk()
                return lst[i]

            def emit_transpose(s, n4):
                # one contiguous block DMA, then transpose 6 column chunks
                xblk = sb.tile([128, C], F32R, tag="xin", name=f"xblk{s}_{n4}", bufs=3)
                if s == 0 and n4 == 0:
                    # split the very first block so transposes start earlier
                    nc.sync.dma_start(xblk[:, 0:384], xs[s, 0:128, 0:384])
                    nc.sync.dma_start(xblk[:, 384:C], xs[s, 0:128, 384:C])
                else:
                    nc.sync.dma_start(xblk[:], xs[s, 128 * n4:128 * (n4 + 1), :])
                for cc in range(CC6):
                    xT = get(xTs[s], cc, lambda cc=cc: sb.tile(
                        [128, N], F32R, tag="xT", name=f"xT_s{s}_{cc}", bufs=8))
                    pt = ps.tile([128, 128], F32R, tag="ps1", name=f"pt{s}_{n4}_{cc}", bufs=8)
                    nc.tensor.transpose(pt[:], xblk[:, 128 * cc:128 * (cc + 1)], ident[:])
                    nc.vector.tensor_copy(xT[:, 128 * n4:128 * (n4 + 1)], pt[:])

            def emit_v(s, n4):
                xT = xTs[s]
                vsb = get(vsbs[s], n4, lambda: sb.tile(
                    [128, H * (Dh + 1)], F32R, tag="vsb", name=f"vsb_s{s}_{n4}", bufs=8))
                pva = ps.tile([128, 512], F32, tag="ps1", name=f"pva{s}_{n4}", bufs=8)
                pvb = ps.tile([128, 256], F32, tag="ps1", name=f"pvb{s}_{n4}", bufs=8)
                for i in range(CC6):
                    cc = (n4 + i) % CC6
                    lhsT = xT[cc][:, 128 * n4:128 * (n4 + 1)]
                    nc.tensor.matmul(pva[:], lhsT, vw[cc][:, 0:512],
                                     start=(i == 0), stop=(i == CC6 - 1))
                    nc.tensor.matmul(pvb[:], lhsT, vw[cc][:, 512:768],
                                     start=(i == 0), stop=(i == CC6 - 1))
                v3 = vsb[:].rearrange("p (h e) -> p h e", e=Dh + 1)
                cpy = nc.scalar.copy if s == 0 else nc.vector.tensor_copy
                cpy(v3[:, 0:8, 0:Dh], pva[:].rearrange("p (h e) -> p h e", e=Dh))
                cpy(v3[:, 8:12, 0:Dh], pvb[:].rearrange("p (h e) -> p h e", e=Dh))
                nc.vector.tensor_copy(v3[:, :, Dh:Dh + 1],
                                      onesf[:, 0:H].rearrange("p (h e) -> p h e", e=1))

            def emit_qk(s, jc):
                xT = xTs[s]
                qkt = get(qks[s], jc, lambda: sb.tile(
                    [128, N], F32R, tag="qk", name=f"qk_s{s}_{jc}", bufs=13))
                pqk = ps.tile([128, N], F32, tag="ps1", name=f"pqk{s}_{jc}", bufs=8)
                for i in range(CC6):
                    cc = (jc + i) % CC6
                    nc.tensor.matmul(pqk[:], qkw[cc][:, 128 * jc:128 * (jc + 1)], xT[cc][:],
                                     start=(i == 0), stop=(i == CC6 - 1))
                nc.vector.tensor_copy(qkt[:], pqk[:])

            def emit_head(s, h):
                qk, vsb = qks[s], vsbs[s]
                hb = 64 * (h % 2)
                qTh = qk[h // 2][hb:hb + 64, :]
                kTh = qk[CC6 + h // 2][hb:hb + 64, :]
                pts = []
                for mc in range(NC4):
                    pst = ps.tile([128, N], F32, tag="ps1", name=f"pst{s}_{h}_{mc}", bufs=8)
                    ptile = sb.tile([128, N], F32R, tag="pt", name=f"ptile{s}_{h}_{mc}", bufs=6)
                    if mc >= 2:
                        # mask added in-PSUM on DVE (PE/DVE load balance)
                        nc.tensor.matmul(pst[:], kTh[:, 128 * mc:128 * (mc + 1)], qTh,
                                         start=True, stop=True)
                        nc.vector.tensor_add(pst[:], pst[:], maskt[mc][:])
                    else:
                        # preload mask into PSUM (sets has_written), scores accumulate
                        nc.tensor.matmul(pst[:], ident[:], maskt[mc][:],
                                         start=True, stop=False, skip_group_check=True)
                        nc.tensor.matmul(pst[:], kTh[:, 128 * mc:128 * (mc + 1)], qTh,
                                         start=False, stop=True, skip_group_check=True)
                    nc.scalar.activation(ptile[:], pst[:],
                                         mybir.ActivationFunctionType.Exp)
                    pts.append(ptile)
                pot = ps.tile([Dh + 1, N], F32, tag="ps1", name=f"pot{s}_{h}", bufs=8)
                for mc in range(NC4):
                    nc.tensor.matmul(pot[:], vsb[mc][:, (Dh + 1) * h:(Dh + 1) * (h + 1)],
                                     pts[mc][:], start=(mc == 0), stop=(mc == NC4 - 1))
                recip = sb.tile([1, N], F32, tag="recip", name=f"recip{s}_{h}", bufs=3)
                nc.vector.reciprocal(recip[:], pot[Dh:Dh + 1, :])
                pbs = sb.tile([Dh, N], F32, tag="pbs", name=f"pbs{s}_{h}", bufs=3)
                nc.gpsimd.partition_broadcast(pbs[:], recip[:], channels=Dh)
                outT = get(outTs[s], h // 2, lambda: sb.tile(
                    [128, N], F32R, tag="outT", name=f"outT_s{s}_{h // 2}", bufs=10))
                with nc.allow_low_precision(reason="f32r outT"):
                    nc.vector.tensor_mul(outT[hb:hb + 64, :], pot[0:Dh, :], pbs[:])

            def emit_proj(s, n4):
                outT = outTs[s]
                if s == 1 and n4 == NC4 - 1:
                    # final unit: 3 narrow psum groups so the drain pipelines
                    osb = sb.tile([128, C], F32, tag="osb", name=f"osb{s}_{n4}", bufs=2)
                    for half in range(3):
                        c0 = 256 * half
                        pr = ps.tile([128, 256], F32, tag="ps1", name=f"pr{s}_{n4}_{half}", bufs=8)
                        for cc in range(CC6):
                            lhsT = outT[cc][:, 128 * n4:128 * (n4 + 1)]
                            nc.tensor.matmul(pr[:], lhsT, projw[cc][:, c0:c0 + 256],
                                             start=(cc == 0), stop=(cc == CC6 - 1))
                        eng = (nc.vector.tensor_copy, nc.scalar.copy)[half % 2]
                        eng(osb[:, c0:c0 + 256], pr[:])
                        deng = (nc.sync, nc.scalar)[half % 2]
                        deng.dma_start(y[s, 128 * n4:128 * (n4 + 1), c0:c0 + 256],
                                       osb[:, c0:c0 + 256])
                    return
                pra = ps.tile([128, 512], F32, tag="ps1", name=f"pra{s}_{n4}", bufs=8)
                prb = ps.tile([128, 256], F32, tag="ps1", name=f"prb{s}_{n4}", bufs=8)
                for cc in range(CC6):
                    lhsT = outT[cc][:, 128 * n4:128 * (n4 + 1)]
                    nc.tensor.matmul(pra[:], lhsT, projw[cc][:, 0:512],
                                     start=(cc == 0), stop=(cc == CC6 - 1))
                    nc.tensor.matmul(prb[:], lhsT, projw[cc][:, 512:768],
                                     start=(cc == 0), stop=(cc == CC6 - 1))
                osb = sb.tile([128, C], F32, tag="osb", name=f"osb{s}_{n4}", bufs=2)
                nc.vector.tensor_copy(osb[:, 0:512], pra[:])
                nc.sync.dma_start(y[s, 128 * n4:128 * (n4 + 1), 0:512], osb[:, 0:512])
                nc.scalar.copy(osb[:, 512:768], prb[:])
                nc.scalar.dma_start(y[s, 128 * n4:128 * (n4 + 1), 512:768], osb[:, 512:768])

            # ---- interleaved schedule ----
            for n4 in range(NC4):
                emit_transpose(0, n4)
            emit_weight_dmas()
            for n4 in range(NC4):
                emit_v(0, n4)
            for jc in range(2 * CC6):
                emit_qk(0, jc)
            # slice 0 attention interleaved with slice 1 early work
            e1 = [(emit_transpose, 1, n4) for n4 in range(NC4)] + \
                 [(emit_v, 1, n4) for n4 in range(NC4)] + \
                 [(emit_qk, 1, jc) for jc in range(2 * CC6)]
            k = 0
            for h in range(H):
                emit_head(0, h)
                if h == 3:
                    emit_projw_dmas()
                tgt = (len(e1) * (h + 1)) // H
                while k < tgt:
                    f, a, b = e1[k]; f(a, b); k += 1
            # slice 1 attention; slice 0 proj folded into the first heads
            p0 = [(emit_proj, 0, n4) for n4 in range(NC4)]
            k = 0
            for h in range(H):
                emit_head(1, h)
                if h < len(p0):
                    f, a, b = p0[k]; f(a, b); k += 1
            for n4 in range(NC4):
                emit_proj(1, n4)

    nc.finalize()
    return nc


def kernel(x, mask, qkv_w, q_bias, v_bias, proj_w, proj_b, _trace=False, _trace_kwargs=None):
    x, mask, qkv_w, proj_w = (np.asarray(a) for a in (x, mask, qkv_w, proj_w))
    q_bias, v_bias, proj_b = (np.asarray(a) for a in (q_bias, v_bias, proj_b))
    scale = Dh ** -0.5
    qkv_wT = np.ascontiguousarray(qkv_w.T).astype(np.float32)
    qkv_wT[:, :C] *= scale
    qkv_wTqk = np.ascontiguousarray(qkv_wT[:, :2 * C])
    qkv_wTv = np.ascontiguousarray(qkv_wT[:, 2 * C:])
    # biases folded in host-side only if nonzero (spec: all zeros). Assert to be safe.
    assert not np.any(q_bias) and not np.any(v_bias) and not np.any(proj_b), \
        "nonzero biases not supported by this kernel build"
    proj_wT = np.ascontiguousarray(proj_w.T).astype(np.float32)
    maskT = np.ascontiguousarray(mask.reshape(N, N).T).astype(np.float32)
    xf = np.ascontiguousarray(x.reshape(B * T, N, C)).astype(np.float32)

    if "nc" not in _cache:
        _cache["nc"] = build_nc()
    nc = _cache["nc"]

    in_maps = []
    for c in range(NCORES):
        in_maps.append({
            "xs": xf[SL * c:SL * (c + 1)],
            "qkv_wTqk": qkv_wTqk,
            "qkv_wTv": qkv_wTv,
            "proj_wT": proj_wT,
            "maskT": maskT,
        })
    res = run_bass_kernel_spmd(
        nc, in_maps, core_ids=list(range(NCORES)),
        trace=_trace, **(_trace_kwargs or {}),
    )
    out = np.concatenate([res.results[c]["y"] for c in range(NCORES)], axis=0)
    out = out.reshape(B, T, N, C)
    if _trace:
        return out, res
    return out



# revision 38
# speedup vs baseline: 1.1898x; 1.1898x over previous
"""Trainium2 Bass kernel for nn_Attention (B=2,T=8,N=512,C=768,H=12).

Strategy: data-parallel over the 16 (b,t) slices -> 2 slices per core, 8 cores.
All math in bf16 on the PE (rel err ~3.5e-3, gate 2e-2). Per slice:
  xT = transpose(x)                      (PE transposes, bf16 identity: 1.0 c/r)
  qkT = Wqk @ xT                         ([d, n] layout; qk scale folded into Wq)
  v   = xT.T @ Wv                        ([token, h*(d|1)] layout, ones column)
  ST[m,n] = kT.T @ qT ; += mask (DVE/Pool) ; P = exp(ST) (ACT, bf16 out)
  pot[n,65] += P[:, nchunk].T @ v_h      (P stationary: 65 rows/matmul, l in col 64)
  out[n,c] = pot * recip(l)              (DVE broadcast mul, bf16)
  outT = transpose(out); y = outT.T @ Wp ([n, C] layout, DMA out f32)
PE rows/slice: 3072 xT + 55296 qkv + 24576 scores + 12480 pv + 3072 outT
+ 18432 proj = 116928 (~48.7us); mask adds and softmax normalize live on
DVE/Pool/ACT which all sit below that.
"""
import sys

sys.path.insert(0, "/opt/trn_rl_repo")

import numpy as np
import ml_dtypes
import concourse.bacc as bacc
import concourse.mybir as mybir
import concourse.tile as tile
from concourse.bass import AP, broadcast_tensor_aps
from concourse.bass_utils import run_bass_kernel_spmd
from concourse.masks import make_identity

B, T, N, C = 2, 8, 512, 768
H = 12
Dh = C // H            # 64
SL = 2                 # slices per core
NCORES = 8
NC4 = N // 128         # 4 n-chunks
CC6 = C // 128         # 6 c-chunks
F32 = mybir.dt.float32
BF16 = mybir.dt.bfloat16
FP8 = mybir.dt.float8e4
EXP = mybir.ActivationFunctionType.Exp
FP8_SCORES = True     # q@k in fp8e4 DoubleRow (2x PE rate); rel err ~1.5e-2
FP8_SCALE = 16.0      # folded into Wq and Wk (256x on scores, undone in exp)

_cache = {}


def build_nc():
    nc = bacc.Bacc()
    xs = nc.dram_tensor("xs", [SL, N, C], BF16, kind="ExternalInput")
    wqk = nc.dram_tensor("wqk", [C, 2 * C], BF16, kind="ExternalInput")
    wv = nc.dram_tensor("wv", [C, C], BF16, kind="ExternalInput")
    wp = nc.dram_tensor("wp", [C, C], BF16, kind="ExternalInput")
    maskT = nc.dram_tensor("maskT", [N, N], F32, kind="ExternalInput")
    y = nc.dram_tensor("y", [SL, N, C], F32, kind="ExternalOutput")

    with tile.TileContext(nc) as tc:
        with (
            tc.tile_pool(name="wpool", bufs=1) as wpool,
            tc.tile_pool(name="sb", bufs=1) as sb,
            tc.tile_pool(name="ps", bufs=1, space="PSUM") as ps,
        ):
            # ---- persistent weights ----
            qkw = [wpool.tile([128, 2 * C], BF16, tag=f"qkw{cc}", name=f"qkw{cc}") for cc in range(CC6)]
            vw = [wpool.tile([128, C], BF16, tag=f"vw{cc}", name=f"vw{cc}") for cc in range(CC6)]
            projw = [wpool.tile([128, C], BF16, tag=f"projw{cc}", name=f"projw{cc}") for cc in range(CC6)]
            maskt = [wpool.tile([128, N], F32, tag=f"maskt{mc}", name=f"maskt{mc}") for mc in range(NC4)]

            def emit_weight_dmas():
                for cc in range(CC6):
                    nc.gpsimd.dma_start(vw[cc][:], wv[128 * cc:128 * (cc + 1), :])
                for cc in range(CC6):
                    nc.gpsimd.dma_start(qkw[cc][:], wqk[128 * cc:128 * (cc + 1), :])

            expm = [wpool.tile([128, N], BF16, tag=f"expm{mc}", name=f"expm{mc}")
                    for mc in range(NC4)]

            def emit_mask_dmas():
                for mc in range(NC4):
                    nc.sync.dma_start(maskt[mc][:], maskT[128 * mc:128 * (mc + 1), :])
                    nc.scalar.activation(expm[mc][:], maskt[mc][:], EXP)

            def emit_projw_dmas():
                for cc in range(CC6):
                    nc.gpsimd.dma_start(projw[cc][:], wp[128 * cc:128 * (cc + 1), :])

            identf = wpool.tile([128, 128], F32, tag="identf", name="identf")
            make_identity(nc, identf[:])
            ident = wpool.tile([128, 128], BF16, tag="ident", name="ident")
            with nc.allow_low_precision(reason="bf16 identity"):
                nc.vector.tensor_copy(ident[:], identf[:])
            onesb = wpool.tile([128, H], BF16, tag="onesb", name="onesb")
            nc.gpsimd.memset(onesb[:], 1.0)

            xTs = [None] * SL
            vsbs = [[None] * NC4 for _ in range(SL)]
            qks = [[None] * (2 * CC6) for _ in range(SL)]
            outs = [[None] * NC4 for _ in range(SL)]
            outTs = [None] * SL
            # psum accumulators: per (slice, head-group): nchunk-pair p holds
            # [n4=2p, 2p+1] x 3 heads x 65 cols (l in col 64); ring of 2
            pots = [None, None]
            cnt = {"cp": 0, "msk": 0}

            def get(lst, i, mk):
                if lst[i] is None:
                    lst[i] = mk()
                return lst[i]

            def cp_eng(i):
                return (nc.vector.tensor_copy, nc.scalar.copy)[i % 2]

            def cp_eng2(i):
                return nc.vector.tensor_copy

            def emit_transpose(s, n4, dve_only=False):
                xblk = sb.tile([128, C], BF16, tag="xin", name=f"xblk{s}_{n4}", bufs=4)
                if s == 0 and n4 == 0:
                    nc.sync.dma_start(xblk[:, 0:384], xs[s, 0:128, 0:384])
                    nc.sync.dma_start(xblk[:, 384:C], xs[s, 0:128, 384:C])
                else:
                    nc.sync.dma_start(xblk[:], xs[s, 128 * n4:128 * (n4 + 1), :])
                xT = get(xTs, s, lambda: sb.tile(
                    [128, CC6 * N], BF16, tag="xT", name=f"xT_s{s}", bufs=2))
                pt = ps.tile([128, 1024], BF16, tag="pt", name=f"pt{s}_{n4}", bufs=1)
                for cc in range(CC6):
                    ptc = pt[:, 128 * cc:128 * (cc + 1)]
                    nc.tensor.transpose(ptc, xblk[:, 128 * cc:128 * (cc + 1)], ident[:])
                    cnt["cp"] += 1
                    ceng = nc.vector.tensor_copy if dve_only else cp_eng2(cnt["cp"])
                    with nc.allow_low_precision(reason="bf16 xT"):
                        ceng(xT[:, N * cc + 128 * n4:N * cc + 128 * (n4 + 1)], ptc)

            def emit_v(s, mc):
                xT = xTs[s]
                vsb = get(vsbs[s], mc, lambda: sb.tile(
                    [128, H * (Dh + 1)], BF16, tag="vsb", name=f"vsb_s{s}_{mc}", bufs=8))
                pva = ps.tile([128, 512], F32, tag="psq", name=f"pva{s}_{mc}", bufs=2)
                pvb = ps.tile([128, 512], F32, tag="psq", name=f"pvb{s}_{mc}", bufs=2)
                for i in range(CC6):
                    cc = (mc + i) % CC6
                    lhsT = xT[:, N * cc + 128 * mc:N * cc + 128 * (mc + 1)]
                    nc.tensor.matmul(pva[:], lhsT, vw[cc][:, 0:512],
                                     start=(i == 0), stop=(i == CC6 - 1))
                    nc.tensor.matmul(pvb[:, 0:256], lhsT, vw[cc][:, 512:768],
                                     start=(i == 0), stop=(i == CC6 - 1))
                v3 = vsb[:].rearrange("p (h e) -> p h e", e=Dh + 1)
                with nc.allow_low_precision(reason="bf16 v"):
                    nc.vector.tensor_copy(v3[:, 0:8, 0:Dh],
                                          pva[:].rearrange("p (h e) -> p h e", e=Dh))
                    nc.scalar.copy(v3[:, 8:12, 0:Dh],
                                   pvb[:, 0:256].rearrange("p (h e) -> p h e", e=Dh))
                    nc.gpsimd.tensor_copy(v3[:, :, Dh:Dh + 1],
                                           onesb[:].rearrange("p (h e) -> p h e", e=1))

            def emit_qk(s, jc):
                xT = xTs[s]
                is_k = jc >= CC6
                if FP8_SCORES and is_k:
                    # k chunk: [128, 2, 512] fp8; plane 1 is the DoubleRow
                    # zero half-contraction (memset once per instance)
                    qkt = get(qks[s], jc, lambda: sb.tile(
                        [128, 2, N], FP8, tag="kf8", name=f"qk_s{s}_{jc}", bufs=7))
                    nc.gpsimd.memset(qkt[:, 1, :], 0.0)
                elif FP8_SCORES:
                    qkt = get(qks[s], jc, lambda: sb.tile(
                        [128, N], FP8, tag="qf8", name=f"qk_s{s}_{jc}", bufs=7))
                else:
                    qkt = get(qks[s], jc, lambda: sb.tile(
                        [128, N], BF16, tag="qk", name=f"qk_s{s}_{jc}", bufs=13))
                pqk = ps.tile([128, N], F32, tag="psq", name=f"pqk{s}_{jc}", bufs=2)
                for i in range(CC6):
                    cc = (jc + i) % CC6
                    nc.tensor.matmul(pqk[:], qkw[cc][:, 128 * jc:128 * (jc + 1)],
                                     xT[:, N * cc:N * (cc + 1)],
                                     start=(i == 0), stop=(i == CC6 - 1))
                cnt["cp"] += 1
                dst = qkt[:, 0, :] if (FP8_SCORES and is_k) else qkt[:]
                with nc.allow_low_precision(reason="low-precision qk"):
                    nc.vector.tensor_copy(dst, pqk[:])

            ptss = {}

            def emit_scores(s, h):
                qk = qks[s]
                hb = 64 * (h % 2)
                if FP8_SCORES:
                    qt = qk[h // 2][hb:hb + 64, :]
                    # rhs [64, 2, 512]: stride-0 dup of q (plane 1 hits zero k)
                    qTh = AP(qt.tensor, qt.offset, [qt.ap[0], [0, 2], qt.ap[1]])
                    kTh = qk[CC6 + h // 2][hb:hb + 64, :, :]
                else:
                    qTh = qk[h // 2][hb:hb + 64, :]
                    kTh = qk[CC6 + h // 2][hb:hb + 64, :]
                pts = []
                for mc in range(NC4):
                    pst = ps.tile([128, N], F32, tag="pst", name=f"pst{s}_{h}_{mc}", bufs=3)
                    ptile = sb.tile([128, N], BF16, tag="ptile", name=f"ptile{s}_{h}_{mc}", bufs=14)
                    if FP8_SCORES:
                        nc.tensor.matmul(pst[:], kTh[:, :, 128 * mc:128 * (mc + 1)],
                                         qTh, start=True, stop=True,
                                         perf_mode=mybir.MatmulPerfMode.DoubleRow)
                    else:
                        nc.tensor.matmul(pst[:], kTh[:, 128 * mc:128 * (mc + 1)], qTh,
                                         start=True, stop=True)
                    nc.scalar.activation(ptile[:], pst[:], EXP,
                                         scale=1.0 / (FP8_SCALE * FP8_SCALE)
                                         if FP8_SCORES else 1.0)
                    cnt["msk"] += 1
                    peng = (nc.vector, nc.gpsimd)[cnt["msk"] % 2]
                    with nc.allow_low_precision(reason="bf16 P"):
                        peng.tensor_mul(ptile[:], ptile[:], expm[mc][:])
                    pts.append(ptile)
                ptss[(s, h)] = pts

            def emit_pv(s, h):
                vsb = vsbs[s]
                g = h // 3          # head group (4 groups of 3)
                hg = h % 3
                pts = ptss.pop((s, h))
                if hg == 0:
                    for p in range(2):
                        pots[p] = ps.tile([128, 2 * 3 * (Dh + 1)], F32, tag="pot",
                                          name=f"pot{s}_{g}_{p}", bufs=2)
                for n4 in range(NC4):
                    pot = pots[n4 // 2][:, 195 * (n4 % 2):195 * (n4 % 2) + 195]
                    for mc in range(NC4):
                        nc.tensor.matmul(pot[:, 65 * hg:65 * (hg + 1)],
                                         pts[mc][:, 128 * n4:128 * (n4 + 1)],
                                         vsb[mc][:, 65 * h:65 * (h + 1)],
                                         start=(mc == 0), stop=(mc == NC4 - 1),
                                         skip_group_check=True)
                if hg == 2:
                    # drain the 3-head group: normalize by row-sums (col 64)
                    for n4 in range(NC4):
                        pot = pots[n4 // 2][:, 195 * (n4 % 2):195 * (n4 % 2) + 195]
                        potv = pot.rearrange("p (h e) -> p h e", e=Dh + 1)
                        lr = sb.tile([128, 3], F32, tag="lr", name=f"lr{s}_{g}_{n4}", bufs=4)
                        nc.vector.reciprocal(
                            lr[:].rearrange("p (h e) -> p h e", e=1),
                            potv[:, :, Dh:Dh + 1])
                        out = get(outs[s], n4, lambda n4=n4: sb.tile(
                            [128, C], BF16, tag="out", name=f"out_s{s}_{n4}", bufs=8))
                        b0, b1 = broadcast_tensor_aps(
                            potv[:, :, 0:Dh],
                            lr[:].rearrange("p (h e) -> p h e", e=1))
                        with nc.allow_low_precision(reason="bf16 out"):
                            nc.vector.tensor_mul(
                                out[:, 192 * g:192 * (g + 1)].rearrange(
                                    "p (h e) -> p h e", e=Dh), b0, b1)

            def emit_outT(s, n4, ccs=None):
                out = outs[s][n4]
                outT = get(outTs, s, lambda: sb.tile(
                    [128, CC6 * N], BF16, tag="outT", name=f"outT_s{s}", bufs=2))
                ccs = range(CC6) if ccs is None else ccs
                pt = ps.tile([128, 1024], BF16, tag="pt",
                             name=f"ot{s}_{n4}_{min(ccs)}", bufs=1)
                for cc in ccs:
                    ptc = pt[:, 128 * cc:128 * (cc + 1)]
                    nc.tensor.transpose(ptc, out[:, 128 * cc:128 * (cc + 1)], ident[:])
                    cnt["cp"] += 1
                    with nc.allow_low_precision(reason="bf16 outT"):
                        cp_eng2(cnt["cp"])(
                            outT[:, N * cc + 128 * n4:N * cc + 128 * (n4 + 1)], ptc)

            def emit_proj(s, n4):
                outT = outTs[s]
                if s == 1 and n4 == NC4 - 1:
                    # final unit: 3 narrow psum groups so the drain pipelines
                    osb = sb.tile([128, C], F32, tag="osb", name=f"osb{s}_{n4}", bufs=2)
                    for third in range(3):
                        c0 = 256 * third
                        pr = ps.tile([128, 512], F32, tag="psq",
                                     name=f"pr{s}_{n4}_{third}", bufs=2)
                        for cc in range(CC6):
                            lhsT = outT[:, N * cc + 128 * n4:N * cc + 128 * (n4 + 1)]
                            nc.tensor.matmul(pr[:, 0:256], lhsT, projw[cc][:, c0:c0 + 256],
                                             start=(cc == 0), stop=(cc == CC6 - 1))
                        ceng = (nc.vector.tensor_copy, nc.scalar.copy,
                                nc.vector.tensor_copy)[third]
                        ceng(osb[:, c0:c0 + 256], pr[:, 0:256])
                        deng = (nc.sync, nc.scalar, nc.sync)[third]
                        deng.dma_start(y[s, 128 * n4:128 * (n4 + 1), c0:c0 + 256],
                                       osb[:, c0:c0 + 256])
                    return
                pra = ps.tile([128, 512], F32, tag="psq", name=f"pra{s}_{n4}", bufs=2)
                prb = ps.tile([128, 512], F32, tag="psq", name=f"prb{s}_{n4}", bufs=2)
                for cc in range(CC6):
                    lhsT = outT[:, N * cc + 128 * n4:N * cc + 128 * (n4 + 1)]
                    nc.tensor.matmul(pra[:], lhsT, projw[cc][:, 0:512],
                                     start=(cc == 0), stop=(cc == CC6 - 1))
                    nc.tensor.matmul(prb[:, 0:256], lhsT, projw[cc][:, 512:768],
                                     start=(cc == 0), stop=(cc == CC6 - 1))
                osb = sb.tile([128, C], F32, tag="osb", name=f"osb{s}_{n4}", bufs=2)
                nc.vector.tensor_copy(osb[:, 0:512], pra[:])
                nc.sync.dma_start(y[s, 128 * n4:128 * (n4 + 1), 0:512], osb[:, 0:512])
                nc.scalar.copy(osb[:, 512:768], prb[:, 0:256])
                nc.sync.dma_start(y[s, 128 * n4:128 * (n4 + 1), 512:768], osb[:, 512:768])

            # ---- interleaved schedule ----
            # scores run one head ahead of PV; qk chunk pairs are emitted on
            # demand right before the scores that need them, so attention
            # starts early and qkv matmuls fill the exp-latency bubbles.
            qk_done = [set(), set()]

            def need_qk(s, h):
                j = h // 2
                if j not in qk_done[s]:
                    qk_done[s].add(j)
                    emit_qk(s, j)
                    emit_qk(s, CC6 + j)

            emit_weight_dmas()
            emit_transpose(0, 0, dve_only=True)
            emit_transpose(0, 1, dve_only=True)
            emit_v(0, 0)
            emit_transpose(0, 2, dve_only=True)
            emit_v(0, 1)
            emit_transpose(0, 3, dve_only=True)
            emit_mask_dmas()
            emit_v(0, 2)
            emit_v(0, 3)
            # slice 0 attention interleaved with slice 1 early work
            e1 = [(emit_transpose, 1, 0), (emit_transpose, 1, 1),
                  (emit_v, 1, 0), (emit_transpose, 1, 2),
                  (emit_v, 1, 1), (emit_transpose, 1, 3),
                  (emit_v, 1, 2), (emit_v, 1, 3)] + \
                 [(emit_qk, 1, jc) for pair in
                  [(j, CC6 + j) for j in range(CC6)] for jc in pair]
            k = 0
            need_qk(0, 0)
            emit_scores(0, 0)
            need_qk(0, 1)
            emit_scores(0, 1)
            for h in range(H):
                if h + 2 < H:
                    need_qk(0, h + 2)
                    emit_scores(0, h + 2)
                emit_pv(0, h)
                if h == 3:
                    emit_projw_dmas()
                tgt = (len(e1) * (h + 1)) // H
                while k < tgt:
                    f, a, b = e1[k]; f(a, b); k += 1
            # slice 1 attention; slice 0 outT+proj folded in
            p0 = [(emit_outT, 0, n4) for n4 in range(NC4)] + \
                 [(emit_proj, 0, n4) for n4 in range(NC4)]
            k = 0
            GRP_CCS = {2: [0], 5: [1, 2], 8: [3], 11: [4, 5]}
            emit_scores(1, 0)
            emit_scores(1, 1)
            for h in range(H):
                if h + 2 < H:
                    emit_scores(1, h + 2)
                emit_pv(1, h)
                if h in GRP_CCS and h != 11:
                    for n4 in range(NC4):
                        emit_outT(1, n4, GRP_CCS[h])
                tgt = (len(p0) * (h + 1)) // H
                while k < tgt:
                    f, a, b = p0[k]; f(a, b); k += 1
            for n4 in range(NC4):
                emit_outT(1, n4, GRP_CCS[11])
                emit_proj(1, n4)

    nc.finalize()
    return nc


def _prep(x, mask, qkv_w, proj_w):
    scale = Dh ** -0.5
    qkv_wT = np.ascontiguousarray(qkv_w.T).astype(np.float32)
    qkv_wT[:, :C] *= scale
    maskT = np.ascontiguousarray(mask.reshape(N, N).T).astype(np.float32)
    if FP8_SCORES:
        # scores computed as (16 Wq x)(16 Wk x) = 256 S; exp scale undoes it
        qkv_wT[:, :2 * C] *= FP8_SCALE
    wqk = qkv_wT[:, :2 * C].astype(ml_dtypes.bfloat16)
    wv = np.ascontiguousarray(qkv_wT[:, 2 * C:]).astype(ml_dtypes.bfloat16)
    wp = np.ascontiguousarray(proj_w.T).astype(np.float32).astype(ml_dtypes.bfloat16)
    xbf = np.ascontiguousarray(x.reshape(B * T, N, C)).astype(ml_dtypes.bfloat16)
    return xbf, wqk, wv, wp, maskT


def make_sim_feed(inputs, core=0):
    """Feed dict for CoreSim replay of core `core` (test.py helper)."""
    x = np.asarray(inputs["x"])
    mask = np.asarray(inputs["mask"])
    qkv_w = np.asarray(inputs["qkv_w"]).astype(np.float32)
    proj_w = np.asarray(inputs["proj_w"]).astype(np.float32)
    xbf, wqk, wv, wp, maskT = _prep(np.asarray(x).astype(np.float32), mask, qkv_w, proj_w)
    return {"xs": xbf[SL * core:SL * (core + 1)], "wqk": wqk, "wv": wv,
            "wp": wp, "maskT": maskT}


def kernel(x, mask, qkv_w, q_bias, v_bias, proj_w, proj_b, _trace=False, _trace_kwargs=None):
    x, mask, qkv_w, proj_w = (np.asarray(a) for a in (x, mask, qkv_w, proj_w))
    q_bias, v_bias, proj_b = (np.asarray(a) for a in (q_bias, v_bias, proj_b))
    # biases folded in host-side only if nonzero (spec: all zeros). Assert to be safe.
    assert not np.any(q_bias) and not np.any(v_bias) and not np.any(proj_b), \
        "nonzero biases not supported by this kernel build"
    xbf, wqk, wv, wp, maskT = _prep(x.astype(np.float32), mask,
                                    qkv_w.astype(np.float32), proj_w.astype(np.float32))

    if "nc" not in _cache:
        _cache["nc"] = build_nc()
    nc = _cache["nc"]

    in_maps = []
    for c in range(NCORES):
        in_maps.append({
            "xs": xbf[SL * c:SL * (c + 1)],
            "wqk": wqk,
            "wv": wv,
            "wp": wp,
            "maskT": maskT,
        })
    res = run_bass_kernel_spmd(
        nc, in_maps, core_ids=list(range(NCORES)),
        trace=_trace, **(_trace_kwargs or {}),
    )
    out = np.concatenate([res.results[c]["y"] for c in range(NCORES)], axis=0)
    out = out.reshape(B, T, N, C)
    if _trace:
        return out, res
    return out


# revision 57
# speedup vs baseline: 1.3131x; 1.1036x over previous
"""Trainium2 Bass kernel for nn_Attention (B=2,T=8,N=512,C=768,H=12).

Strategy: data-parallel over the 16 (b,t) slices -> 2 slices per core, 8 cores.
All math in bf16 on the PE (rel err ~3.5e-3, gate 2e-2). Per slice:
  xT = transpose(x)                      (PE transposes, bf16 identity: 1.0 c/r)
  qkT = Wqk @ xT                         ([d, n] layout; qk scale folded into Wq)
  v   = xT.T @ Wv                        ([token, h*(d|1)] layout, ones column)
  ST[m,n] = kT.T @ qT ; += mask (DVE/Pool) ; P = exp(ST) (ACT, bf16 out)
  pot[n,65] += P[:, nchunk].T @ v_h      (P stationary: 65 rows/matmul, l in col 64)
  out[n,c] = pot * recip(l)              (DVE broadcast mul, bf16)
  outT = transpose(out); y = outT.T @ Wp ([n, C] layout, DMA out f32)
PE rows/slice: 3072 xT + 55296 qkv + 24576 scores + 12480 pv + 3072 outT
+ 18432 proj = 116928 (~48.7us); mask adds and softmax normalize live on
DVE/Pool/ACT which all sit below that.
"""
import sys

sys.path.insert(0, "/opt/trn_rl_repo")

import numpy as np
import ml_dtypes
import concourse.bacc as bacc
import concourse.mybir as mybir
import concourse.tile as tile
from concourse.bass import AP, broadcast_tensor_aps
from concourse.bass_utils import run_bass_kernel_spmd
from concourse.masks import make_identity

B, T, N, C = 2, 8, 512, 768
H = 12
Dh = C // H            # 64
SL = 2                 # slices per core
NCORES = 8
NC4 = N // 128         # 4 n-chunks
CC6 = C // 128         # 6 c-chunks
F32 = mybir.dt.float32
BF16 = mybir.dt.bfloat16
FP8 = mybir.dt.float8e4
EXP = mybir.ActivationFunctionType.Exp
FP8_SCORES = True     # q@k in fp8e4 DoubleRow (2x PE rate); rel err ~1.5e-2
FP8_SCALE = 16.0      # folded into Wq and Wk (256x on scores, undone in exp)

_cache = {}


def build_nc():
    nc = bacc.Bacc()
    xs = nc.dram_tensor("xs", [SL, N, C], BF16, kind="ExternalInput")
    wqk = nc.dram_tensor("wqk", [C, 2 * C], BF16, kind="ExternalInput")
    wv = nc.dram_tensor("wv", [C, C], BF16, kind="ExternalInput")
    wp = nc.dram_tensor("wp", [C, C], BF16, kind="ExternalInput")
    maskT = nc.dram_tensor("maskT", [N, N], F32, kind="ExternalInput")
    y = nc.dram_tensor("y", [SL, N, C], F32, kind="ExternalOutput")

    with tile.TileContext(nc) as tc:
        with (
            tc.tile_pool(name="wpool", bufs=1) as wpool,
            tc.tile_pool(name="sb", bufs=1) as sb,
            tc.tile_pool(name="ps", bufs=1, space="PSUM") as ps,
        ):
            # ---- persistent weights ----
            qkw = [wpool.tile([128, 2 * C], BF16, tag=f"qkw{cc}", name=f"qkw{cc}") for cc in range(CC6)]
            vw = [wpool.tile([128, C], BF16, tag=f"vw{cc}", name=f"vw{cc}") for cc in range(CC6)]
            projw = [wpool.tile([128, C], BF16, tag=f"projw{cc}", name=f"projw{cc}") for cc in range(CC6)]
            maskt = [wpool.tile([128, N], F32, tag=f"maskt{mc}", name=f"maskt{mc}") for mc in range(NC4)]

            def emit_weight_dmas():
                for cc in range(CC6):
                    nc.gpsimd.dma_start(vw[cc][:], wv[128 * cc:128 * (cc + 1), :])
                for cc in range(CC6):
                    nc.gpsimd.dma_start(qkw[cc][:], wqk[128 * cc:128 * (cc + 1), :])

            expm = [wpool.tile([128, N], BF16, tag=f"expm{mc}", name=f"expm{mc}")
                    for mc in range(NC4)]

            def emit_mask_dmas():
                for mc in range(NC4):
                    nc.sync.dma_start(maskt[mc][:], maskT[128 * mc:128 * (mc + 1), :])
                    nc.scalar.activation(expm[mc][:], maskt[mc][:], EXP)

            def emit_projw_dmas():
                for cc in range(CC6):
                    nc.gpsimd.dma_start(projw[cc][:], wp[128 * cc:128 * (cc + 1), :])

            identf = wpool.tile([128, 128], F32, tag="identf", name="identf")
            make_identity(nc, identf[:])
            ident = wpool.tile([128, 128], BF16, tag="ident", name="ident")
            with nc.allow_low_precision(reason="bf16 identity"):
                nc.vector.tensor_copy(ident[:], identf[:])
            onesb = wpool.tile([128, H], BF16, tag="onesb", name="onesb")
            nc.gpsimd.memset(onesb[:], 1.0)

            xTs = [None] * SL
            vsbs = [[None] * NC4 for _ in range(SL)]
            qks = [[None] * (2 * CC6) for _ in range(SL)]
            outs = [[None] * NC4 for _ in range(SL)]
            outTs = [None] * SL
            # psum accumulators: per (slice, head-group): nchunk-pair p holds
            # [n4=2p, 2p+1] x 3 heads x 65 cols (l in col 64); ring of 2
            pots = [None, None]
            cnt = {"cp": 0, "msk": 0}

            def get(lst, i, mk):
                if lst[i] is None:
                    lst[i] = mk()
                return lst[i]

            def cp_eng(i):
                return (nc.vector.tensor_copy, nc.scalar.copy)[i % 2]

            def cp_eng2(i):
                return nc.vector.tensor_copy

            def emit_transpose(s, n4, dve_only=False):
                xblk = sb.tile([128, C], BF16, tag="xin", name=f"xblk{s}_{n4}", bufs=4)
                if s == 0 and n4 == 0:
                    nc.sync.dma_start(xblk[:, 0:384], xs[s, 0:128, 0:384])
                    nc.sync.dma_start(xblk[:, 384:C], xs[s, 0:128, 384:C])
                else:
                    nc.sync.dma_start(xblk[:], xs[s, 128 * n4:128 * (n4 + 1), :])
                xT = get(xTs, s, lambda: sb.tile(
                    [128, CC6 * N], BF16, tag="xT", name=f"xT_s{s}", bufs=2))
                pt = ps.tile([128, 1024], BF16, tag="pst", name=f"pt{s}_{n4}", bufs=3)
                for cc in range(CC6):
                    ptc = pt[:, 128 * cc:128 * (cc + 1)]
                    nc.tensor.transpose(ptc, xblk[:, 128 * cc:128 * (cc + 1)], ident[:])
                    cnt["cp"] += 1
                    ceng = nc.vector.tensor_copy if dve_only else cp_eng2(cnt["cp"])
                    with nc.allow_low_precision(reason="bf16 xT"):
                        ceng(xT[:, N * cc + 128 * n4:N * cc + 128 * (n4 + 1)], ptc)

            def emit_v(s, mc):
                xT = xTs[s]
                vsb = get(vsbs[s], mc, lambda: sb.tile(
                    [128, H * (Dh + 1)], BF16, tag="vsb", name=f"vsb_s{s}_{mc}", bufs=8))
                pva = ps.tile([128, 512], F32, tag="psq", name=f"pva{s}_{mc}", bufs=3)
                pvb = ps.tile([128, 512], F32, tag="psq", name=f"pvb{s}_{mc}", bufs=3)
                for i in range(CC6):
                    cc = (mc + i) % CC6
                    lhsT = xT[:, N * cc + 128 * mc:N * cc + 128 * (mc + 1)]
                    nc.tensor.matmul(pva[:], lhsT, vw[cc][:, 0:512],
                                     start=(i == 0), stop=(i == CC6 - 1))
                    nc.tensor.matmul(pvb[:, 0:256], lhsT, vw[cc][:, 512:768],
                                     start=(i == 0), stop=(i == CC6 - 1))
                v3 = vsb[:].rearrange("p (h e) -> p h e", e=Dh + 1)
                with nc.allow_low_precision(reason="bf16 v"):
                    nc.vector.tensor_copy(v3[:, 0:8, 0:Dh],
                                          pva[:].rearrange("p (h e) -> p h e", e=Dh))
                    nc.scalar.copy(v3[:, 8:12, 0:Dh],
                                   pvb[:, 0:256].rearrange("p (h e) -> p h e", e=Dh))
                    nc.gpsimd.tensor_copy(v3[:, :, Dh:Dh + 1],
                                           onesb[:].rearrange("p (h e) -> p h e", e=1))

            def emit_qk(s, jc):
                xT = xTs[s]
                is_k = jc >= CC6
                if FP8_SCORES and is_k:
                    # k chunk: [128, 2, 512] fp8; plane 1 is the DoubleRow
                    # zero half-contraction (memset once per instance)
                    qkt = get(qks[s], jc, lambda: sb.tile(
                        [128, 2, N], FP8, tag="kf8", name=f"qk_s{s}_{jc}", bufs=7))
                    nc.gpsimd.memset(qkt[:, 1, :], 0.0)
                elif FP8_SCORES:
                    qkt = get(qks[s], jc, lambda: sb.tile(
                        [128, N], FP8, tag="qf8", name=f"qk_s{s}_{jc}", bufs=7))
                else:
                    qkt = get(qks[s], jc, lambda: sb.tile(
                        [128, N], BF16, tag="qk", name=f"qk_s{s}_{jc}", bufs=13))
                pqk = ps.tile([128, N], F32, tag="psq", name=f"pqk{s}_{jc}", bufs=3)
                for i in range(CC6):
                    cc = (jc + i) % CC6
                    nc.tensor.matmul(pqk[:], qkw[cc][:, 128 * jc:128 * (jc + 1)],
                                     xT[:, N * cc:N * (cc + 1)],
                                     start=(i == 0), stop=(i == CC6 - 1))
                cnt["cp"] += 1
                dst = qkt[:, 0, :] if (FP8_SCORES and is_k) else qkt[:]
                with nc.allow_low_precision(reason="low-precision qk"):
                    nc.vector.tensor_copy(dst, pqk[:])

            ptss = {}

            def emit_scores(s, h):
                qk = qks[s]
                hb = 64 * (h % 2)
                if FP8_SCORES:
                    qt = qk[h // 2][hb:hb + 64, :]
                    # rhs [64, 2, 512]: stride-0 dup of q (plane 1 hits zero k)
                    qTh = AP(qt.tensor, qt.offset, [qt.ap[0], [0, 2], qt.ap[1]])
                    kTh = qk[CC6 + h // 2][hb:hb + 64, :, :]
                else:
                    qTh = qk[h // 2][hb:hb + 64, :]
                    kTh = qk[CC6 + h // 2][hb:hb + 64, :]
                pts = []
                for mc in range(NC4):
                    pst = ps.tile([128, N], F32, tag="pst", name=f"pst{s}_{h}_{mc}", bufs=3)
                    ptile = sb.tile([128, N], BF16, tag="ptile", name=f"ptile{s}_{h}_{mc}", bufs=26)
                    if FP8_SCORES:
                        nc.tensor.matmul(pst[:], kTh[:, :, 128 * mc:128 * (mc + 1)],
                                         qTh, start=True, stop=True,
                                         perf_mode=mybir.MatmulPerfMode.DoubleRow)
                    else:
                        nc.tensor.matmul(pst[:], kTh[:, 128 * mc:128 * (mc + 1)], qTh,
                                         start=True, stop=True)
                    nc.scalar.activation(ptile[:], pst[:], EXP,
                                         scale=1.0 / (FP8_SCALE * FP8_SCALE)
                                         if FP8_SCORES else 1.0)
                    cnt["msk"] += 1
                    peng = (nc.vector, nc.gpsimd)[cnt["msk"] % 2]
                    with nc.allow_low_precision(reason="bf16 P"):
                        peng.tensor_mul(ptile[:], ptile[:], expm[mc][:])
                    pts.append(ptile)
                ptss[(s, h)] = pts

            def emit_pv(s, h):
                vsb = vsbs[s]
                g = h // 3          # head group (4 groups of 3)
                hg = h % 3
                pts = ptss.pop((s, h))
                if hg == 0:
                    for p in range(2):
                        pots[p] = ps.tile([128, 2 * 3 * (Dh + 1)], F32, tag="pot",
                                          name=f"pot{s}_{g}_{p}", bufs=2)
                for n4 in range(NC4):
                    pot = pots[n4 // 2][:, 195 * (n4 % 2):195 * (n4 % 2) + 195]
                    for mc in range(NC4):
                        nc.tensor.matmul(pot[:, 65 * hg:65 * (hg + 1)],
                                         pts[mc][:, 128 * n4:128 * (n4 + 1)],
                                         vsb[mc][:, 65 * h:65 * (h + 1)],
                                         start=(mc == 0), stop=(mc == NC4 - 1),
                                         skip_group_check=True)
                if hg == 2:
                    # drain the 3-head group: normalize by row-sums (col 64)
                    for n4 in range(NC4):
                        pot = pots[n4 // 2][:, 195 * (n4 % 2):195 * (n4 % 2) + 195]
                        potv = pot.rearrange("p (h e) -> p h e", e=Dh + 1)
                        lr = sb.tile([128, 3], F32, tag="lr", name=f"lr{s}_{g}_{n4}", bufs=4)
                        nc.vector.reciprocal(
                            lr[:].rearrange("p (h e) -> p h e", e=1),
                            potv[:, :, Dh:Dh + 1])
                        out = get(outs[s], n4, lambda n4=n4: sb.tile(
                            [128, C], BF16, tag="out", name=f"out_s{s}_{n4}", bufs=8))
                        b0, b1 = broadcast_tensor_aps(
                            potv[:, :, 0:Dh],
                            lr[:].rearrange("p (h e) -> p h e", e=1))
                        with nc.allow_low_precision(reason="bf16 out"):
                            nc.vector.tensor_mul(
                                out[:, 192 * g:192 * (g + 1)].rearrange(
                                    "p (h e) -> p h e", e=Dh), b0, b1)

            def emit_outT(s, n4, ccs=None):
                out = outs[s][n4]
                outT = get(outTs, s, lambda: sb.tile(
                    [128, CC6 * N], BF16, tag="outT", name=f"outT_s{s}", bufs=2))
                ccs = range(CC6) if ccs is None else ccs
                pt = ps.tile([128, 1024], BF16, tag="pst",
                             name=f"ot{s}_{n4}_{min(ccs)}", bufs=3)
                for cc in ccs:
                    ptc = pt[:, 128 * cc:128 * (cc + 1)]
                    nc.tensor.transpose(ptc, out[:, 128 * cc:128 * (cc + 1)], ident[:])
                    cnt["cp"] += 1
                    with nc.allow_low_precision(reason="bf16 outT"):
                        cp_eng2(cnt["cp"])(
                            outT[:, N * cc + 128 * n4:N * cc + 128 * (n4 + 1)], ptc)

            def emit_proj(s, n4):
                outT = outTs[s]
                if s == 1 and n4 == NC4 - 1:
                    # final unit: 3 narrow psum groups so the drain pipelines
                    osb = sb.tile([128, C], F32, tag="osb", name=f"osb{s}_{n4}", bufs=2)
                    bounds = [(0, 320), (320, 640), (640, 768)]
                    for third, (c0, c1) in enumerate(bounds):
                        w = c1 - c0
                        pr = ps.tile([128, 512], F32, tag="psq",
                                     name=f"pr{s}_{n4}_{third}", bufs=3)
                        for cc in range(CC6):
                            lhsT = outT[:, N * cc + 128 * n4:N * cc + 128 * (n4 + 1)]
                            nc.tensor.matmul(pr[:, 0:w], lhsT, projw[cc][:, c0:c1],
                                             start=(cc == 0), stop=(cc == CC6 - 1))
                        ceng = (nc.vector.tensor_copy, nc.scalar.copy,
                                nc.vector.tensor_copy)[third]
                        ceng(osb[:, c0:c1], pr[:, 0:w])
                        deng = (nc.sync, nc.scalar, nc.sync)[third]
                        deng.dma_start(y[s, 128 * n4:128 * (n4 + 1), c0:c1],
                                       osb[:, c0:c1])
                    return
                pra = ps.tile([128, 512], F32, tag="psq", name=f"pra{s}_{n4}", bufs=3)
                prb = ps.tile([128, 512], F32, tag="psq", name=f"prb{s}_{n4}", bufs=3)
                for cc in range(CC6):
                    lhsT = outT[:, N * cc + 128 * n4:N * cc + 128 * (n4 + 1)]
                    nc.tensor.matmul(pra[:], lhsT, projw[cc][:, 0:512],
                                     start=(cc == 0), stop=(cc == CC6 - 1))
                    nc.tensor.matmul(prb[:, 0:256], lhsT, projw[cc][:, 512:768],
                                     start=(cc == 0), stop=(cc == CC6 - 1))
                osb = sb.tile([128, C], F32, tag="osb", name=f"osb{s}_{n4}", bufs=2)
                nc.vector.tensor_copy(osb[:, 0:512], pra[:])
                nc.sync.dma_start(y[s, 128 * n4:128 * (n4 + 1), 0:512], osb[:, 0:512])
                nc.scalar.copy(osb[:, 512:768], prb[:, 0:256])
                nc.sync.dma_start(y[s, 128 * n4:128 * (n4 + 1), 512:768], osb[:, 512:768])

            # ---- interleaved schedule ----
            # scores run two heads ahead of PV; qk chunk pairs are emitted on
            # demand right before the scores that need them, so attention
            # starts early and qkv matmuls fill the exp-latency bubbles.

            def need_qk(s, h):
                j = h // 2
                if j not in qk_done[s]:
                    qk_done[s].add(j)
                    emit_qk(s, j)
                    emit_qk(s, CC6 + j)

            qk_done = [set(), set()]
            emit_weight_dmas()
            emit_transpose(0, 0, dve_only=True)
            emit_transpose(0, 1, dve_only=True)
            emit_v(0, 0)
            emit_transpose(0, 2, dve_only=True)
            emit_v(0, 1)
            emit_transpose(0, 3, dve_only=True)
            emit_mask_dmas()
            emit_v(0, 2)
            emit_v(0, 3)
            GRP_CCS0 = {2: [0], 5: [1, 2], 8: [3], 11: [4, 5]}
            # slice 0 attention interleaved with slice 1 early work
            e1 = [(emit_transpose, 1, 0), (emit_transpose, 1, 1),
                  (emit_v, 1, 0), (emit_transpose, 1, 2),
                  (emit_v, 1, 1), (emit_transpose, 1, 3),
                  (emit_v, 1, 2), (emit_v, 1, 3)] + \
                 [(emit_qk, 1, jc) for pair in
                  [(j, CC6 + j) for j in range(CC6)] for jc in pair]
            k = 0
            for hh in range(6):
                need_qk(0, hh)
                emit_scores(0, hh)
            for h in range(H):
                if h + 6 < H:
                    need_qk(0, h + 6)
                    emit_scores(0, h + 6)
                else:
                    # lookahead crosses into slice 1 so the exp stream
                    # never drains at the slice boundary
                    emit_scores(1, h + 6 - H)
                emit_pv(0, h)
                if h == 3:
                    emit_projw_dmas()
                tgt = (len(e1) * (h + 1)) // H
                while k < tgt:
                    f, a, b = e1[k]; f(a, b); k += 1
            # slice 1 attention; slice 0 outT+proj folded in
            p0 = [(emit_outT, 0, n4) for n4 in range(NC4)] + \
                 [(emit_proj, 0, n4) for n4 in range(NC4)]
            k = 0
            GRP_CCS = {2: [0], 5: [1, 2], 8: [3], 11: [4, 5]}
            for h in range(H):
                if h + 6 < H:
                    emit_scores(1, h + 6)
                emit_pv(1, h)
                if h in GRP_CCS and h != 11:
                    for n4 in range(NC4):
                        emit_outT(1, n4, GRP_CCS[h])
                tgt = (len(p0) * (h + 1)) // H
                while k < tgt:
                    u = p0[k]; u[0](*u[1:]); k += 1
            for n4 in range(NC4):
                emit_outT(1, n4, GRP_CCS[11])
                emit_proj(1, n4)

    nc.finalize()
    return nc


def _prep(x, mask, qkv_w, proj_w):
    scale = Dh ** -0.5
    qkv_wT = np.ascontiguousarray(qkv_w.T).astype(np.float32)
    qkv_wT[:, :C] *= scale
    maskT = np.ascontiguousarray(mask.reshape(N, N).T).astype(np.float32)
    if FP8_SCORES:
        # scores computed as (16 Wq x)(16 Wk x) = 256 S; exp scale undoes it
        qkv_wT[:, :2 * C] *= FP8_SCALE
    wqk = qkv_wT[:, :2 * C].astype(ml_dtypes.bfloat16)
    wv = np.ascontiguousarray(qkv_wT[:, 2 * C:]).astype(ml_dtypes.bfloat16)
    wp = np.ascontiguousarray(proj_w.T).astype(np.float32).astype(ml_dtypes.bfloat16)
    xbf = np.ascontiguousarray(x.reshape(B * T, N, C)).astype(ml_dtypes.bfloat16)
    return xbf, wqk, wv, wp, maskT


def make_sim_feed(inputs, core=0):
    """Feed dict for CoreSim replay of core `core` (test.py helper)."""
    x = np.asarray(inputs["x"])
    mask = np.asarray(inputs["mask"])
    qkv_w = np.asarray(inputs["qkv_w"]).astype(np.float32)
    proj_w = np.asarray(inputs["proj_w"]).astype(np.float32)
    xbf, wqk, wv, wp, maskT = _prep(np.asarray(x).astype(np.float32), mask, qkv_w, proj_w)
    return {"xs": xbf[SL * core:SL * (core + 1)], "wqk": wqk, "wv": wv,
            "wp": wp, "maskT": maskT}


def kernel(x, mask, qkv_w, q_bias, v_bias, proj_w, proj_b, _trace=False, _trace_kwargs=None):
    x, mask, qkv_w, proj_w = (np.asarray(a) for a in (x, mask, qkv_w, proj_w))
    q_bias, v_bias, proj_b = (np.asarray(a) for a in (q_bias, v_bias, proj_b))
    # biases folded in host-side only if nonzero (spec: all zeros). Assert to be safe.
    assert not np.any(q_bias) and not np.any(v_bias) and not np.any(proj_b), \
        "nonzero biases not supported by this kernel build"
    xbf, wqk, wv, wp, maskT = _prep(x.astype(np.float32), mask,
                                    qkv_w.astype(np.float32), proj_w.astype(np.float32))

    if "nc" not in _cache:
        _cache["nc"] = build_nc()
    nc = _cache["nc"]

    in_maps = []
    for c in range(NCORES):
        in_maps.append({
            "xs": xbf[SL * c:SL * (c + 1)],
            "wqk": wqk,
            "wv": wv,
            "wp": wp,
            "maskT": maskT,
        })
    res = run_bass_kernel_spmd(
        nc, in_maps, core_ids=list(range(NCORES)),
        trace=_trace, **(_trace_kwargs or {}),
    )
    out = np.concatenate([res.results[c]["y"] for c in range(NCORES)], axis=0)
    out = out.reshape(B, T, N, C)
    if _trace:
        return out, res
    return out


# revision 62
# speedup vs baseline: 1.3151x; 1.0015x over previous
"""Trainium2 Bass kernel for nn_Attention (B=2,T=8,N=512,C=768,H=12).

Strategy: data-parallel over the 16 (b,t) slices -> 2 slices per core, 8 cores.
All math in bf16 on the PE (rel err ~3.5e-3, gate 2e-2). Per slice:
  xT = transpose(x)                      (PE transposes, bf16 identity: 1.0 c/r)
  qkT = Wqk @ xT                         ([d, n] layout; qk scale folded into Wq)
  v   = xT.T @ Wv                        ([token, h*(d|1)] layout, ones column)
  ST[m,n] = kT.T @ qT ; += mask (DVE/Pool) ; P = exp(ST) (ACT, bf16 out)
  pot[n,65] += P[:, nchunk].T @ v_h      (P stationary: 65 rows/matmul, l in col 64)
  out[n,c] = pot * recip(l)              (DVE broadcast mul, bf16)
  outT = transpose(out); y = outT.T @ Wp ([n, C] layout, DMA out f32)
PE rows/slice: 3072 xT + 55296 qkv + 24576 scores + 12480 pv + 3072 outT
+ 18432 proj = 116928 (~48.7us); mask adds and softmax normalize live on
DVE/Pool/ACT which all sit below that.
"""
import sys

sys.path.insert(0, "/opt/trn_rl_repo")

import numpy as np
import ml_dtypes
import concourse.bacc as bacc
import concourse.mybir as mybir
import concourse.tile as tile
from concourse.bass import AP, broadcast_tensor_aps
from concourse.bass_utils import run_bass_kernel_spmd
from concourse.masks import make_identity

B, T, N, C = 2, 8, 512, 768
H = 12
Dh = C // H            # 64
SL = 2                 # slices per core
NCORES = 8
NC4 = N // 128         # 4 n-chunks
CC6 = C // 128         # 6 c-chunks
F32 = mybir.dt.float32
BF16 = mybir.dt.bfloat16
FP8 = mybir.dt.float8e4
EXP = mybir.ActivationFunctionType.Exp
FP8_SCORES = True     # q@k in fp8e4 DoubleRow (2x PE rate); rel err ~1.5e-2
FP8_SCALE = 16.0      # folded into Wq and Wk (256x on scores, undone in exp)

_cache = {}


def build_nc():
    nc = bacc.Bacc()
    xs = nc.dram_tensor("xs", [SL, N, C], BF16, kind="ExternalInput")
    wqk = nc.dram_tensor("wqk", [C, 2 * C], BF16, kind="ExternalInput")
    wv = nc.dram_tensor("wv", [C, C], BF16, kind="ExternalInput")
    wp = nc.dram_tensor("wp", [C, C], BF16, kind="ExternalInput")
    maskT = nc.dram_tensor("maskT", [N, N], F32, kind="ExternalInput")
    y = nc.dram_tensor("y", [SL, N, C], F32, kind="ExternalOutput")

    with tile.TileContext(nc) as tc:
        with (
            tc.tile_pool(name="wpool", bufs=1) as wpool,
            tc.tile_pool(name="sb", bufs=1) as sb,
            tc.tile_pool(name="ps", bufs=1, space="PSUM") as ps,
        ):
            # ---- persistent weights ----
            qkw = [wpool.tile([128, 2 * C], BF16, tag=f"qkw{cc}", name=f"qkw{cc}") for cc in range(CC6)]
            vw = [wpool.tile([128, C], BF16, tag=f"vw{cc}", name=f"vw{cc}") for cc in range(CC6)]
            projw = [wpool.tile([128, C], BF16, tag=f"projw{cc}", name=f"projw{cc}") for cc in range(CC6)]
            maskt = [wpool.tile([128, N], F32, tag=f"maskt{mc}", name=f"maskt{mc}") for mc in range(NC4)]

            def emit_weight_dmas():
                for cc in range(CC6):
                    nc.gpsimd.dma_start(vw[cc][:], wv[128 * cc:128 * (cc + 1), :])
                for cc in range(CC6):
                    nc.gpsimd.dma_start(qkw[cc][:], wqk[128 * cc:128 * (cc + 1), :])

            expm = [wpool.tile([128, N], BF16, tag=f"expm{mc}", name=f"expm{mc}")
                    for mc in range(NC4)]

            def emit_mask_dmas():
                for mc in range(NC4):
                    nc.sync.dma_start(maskt[mc][:], maskT[128 * mc:128 * (mc + 1), :])
                    nc.scalar.activation(expm[mc][:], maskt[mc][:], EXP)

            def emit_projw_dmas():
                for cc in range(CC6):
                    nc.gpsimd.dma_start(projw[cc][:], wp[128 * cc:128 * (cc + 1), :])

            identf = wpool.tile([128, 128], F32, tag="identf", name="identf")
            make_identity(nc, identf[:])
            ident = wpool.tile([128, 128], BF16, tag="ident", name="ident")
            with nc.allow_low_precision(reason="bf16 identity"):
                nc.vector.tensor_copy(ident[:], identf[:])
            onesb = wpool.tile([128, H], BF16, tag="onesb", name="onesb")
            nc.gpsimd.memset(onesb[:], 1.0)

            xTs = [None] * SL
            vsbs = [[None] * NC4 for _ in range(SL)]
            qks = [[None] * (2 * CC6) for _ in range(SL)]
            outs = [[None] * NC4 for _ in range(SL)]
            outTs = [None] * SL
            # psum accumulators: per (slice, head-group): nchunk-pair p holds
            # [n4=2p, 2p+1] x 3 heads x 65 cols (l in col 64); ring of 2
            pots = [None, None]
            cnt = {"cp": 0, "msk": 0}

            def get(lst, i, mk):
                if lst[i] is None:
                    lst[i] = mk()
                return lst[i]

            def cp_eng(i):
                return (nc.vector.tensor_copy, nc.scalar.copy)[i % 2]

            def cp_eng2(i):
                return nc.vector.tensor_copy

            def emit_transpose(s, n4, dve_only=False):
                xblk = sb.tile([128, C], BF16, tag="xin", name=f"xblk{s}_{n4}", bufs=4)
                if s == 0 and n4 == 0:
                    nc.sync.dma_start(xblk[:, 0:384], xs[s, 0:128, 0:384])
                    nc.sync.dma_start(xblk[:, 384:C], xs[s, 0:128, 384:C])
                else:
                    nc.sync.dma_start(xblk[:], xs[s, 128 * n4:128 * (n4 + 1), :])
                xT = get(xTs, s, lambda: sb.tile(
                    [128, CC6 * N], BF16, tag="xT", name=f"xT_s{s}", bufs=2))
                pt = ps.tile([128, 1024], BF16, tag="pst", name=f"pt{s}_{n4}", bufs=3)
                for cc in range(CC6):
                    ptc = pt[:, 128 * cc:128 * (cc + 1)]
                    nc.tensor.transpose(ptc, xblk[:, 128 * cc:128 * (cc + 1)], ident[:])
                    cnt["cp"] += 1
                    ceng = nc.vector.tensor_copy if dve_only else cp_eng2(cnt["cp"])
                    with nc.allow_low_precision(reason="bf16 xT"):
                        ceng(xT[:, N * cc + 128 * n4:N * cc + 128 * (n4 + 1)], ptc)

            def emit_v(s, mc):
                xT = xTs[s]
                vsb = get(vsbs[s], mc, lambda: sb.tile(
                    [128, H * (Dh + 1)], BF16, tag="vsb", name=f"vsb_s{s}_{mc}", bufs=8))
                pva = ps.tile([128, 512], F32, tag="psq", name=f"pva{s}_{mc}", bufs=3)
                pvb = ps.tile([128, 512], F32, tag="psq", name=f"pvb{s}_{mc}", bufs=3)
                for i in range(CC6):
                    cc = (mc + i) % CC6
                    lhsT = xT[:, N * cc + 128 * mc:N * cc + 128 * (mc + 1)]
                    nc.tensor.matmul(pva[:], lhsT, vw[cc][:, 0:512],
                                     start=(i == 0), stop=(i == CC6 - 1))
                    nc.tensor.matmul(pvb[:, 0:256], lhsT, vw[cc][:, 512:768],
                                     start=(i == 0), stop=(i == CC6 - 1))
                v3 = vsb[:].rearrange("p (h e) -> p h e", e=Dh + 1)
                with nc.allow_low_precision(reason="bf16 v"):
                    nc.vector.tensor_copy(v3[:, 0:8, 0:Dh],
                                          pva[:].rearrange("p (h e) -> p h e", e=Dh))
                    nc.scalar.copy(v3[:, 8:12, 0:Dh],
                                   pvb[:, 0:256].rearrange("p (h e) -> p h e", e=Dh))
                    nc.gpsimd.tensor_copy(v3[:, :, Dh:Dh + 1],
                                           onesb[:].rearrange("p (h e) -> p h e", e=1))

            def emit_qk(s, jc):
                xT = xTs[s]
                is_k = jc >= CC6
                if FP8_SCORES and is_k:
                    # k chunk: [128, 2, 512] fp8; plane 1 is the DoubleRow
                    # zero half-contraction (memset once per instance)
                    qkt = get(qks[s], jc, lambda: sb.tile(
                        [128, 2, N], FP8, tag="kf8", name=f"qk_s{s}_{jc}", bufs=7))
                    nc.gpsimd.memset(qkt[:, 1, :], 0.0)
                elif FP8_SCORES:
                    qkt = get(qks[s], jc, lambda: sb.tile(
                        [128, N], FP8, tag="qf8", name=f"qk_s{s}_{jc}", bufs=7))
                else:
                    qkt = get(qks[s], jc, lambda: sb.tile(
                        [128, N], BF16, tag="qk", name=f"qk_s{s}_{jc}", bufs=13))
                pqk = ps.tile([128, N], F32, tag="psq", name=f"pqk{s}_{jc}", bufs=3)
                for i in range(CC6):
                    cc = (jc + i) % CC6
                    nc.tensor.matmul(pqk[:], qkw[cc][:, 128 * jc:128 * (jc + 1)],
                                     xT[:, N * cc:N * (cc + 1)],
                                     start=(i == 0), stop=(i == CC6 - 1))
                cnt["cp"] += 1
                dst = qkt[:, 0, :] if (FP8_SCORES and is_k) else qkt[:]
                with nc.allow_low_precision(reason="low-precision qk"):
                    nc.vector.tensor_copy(dst, pqk[:])

            ptss = {}

            def emit_scores(s, h):
                qk = qks[s]
                hb = 64 * (h % 2)
                if FP8_SCORES:
                    qt = qk[h // 2][hb:hb + 64, :]
                    # rhs [64, 2, 512]: stride-0 dup of q (plane 1 hits zero k)
                    qTh = AP(qt.tensor, qt.offset, [qt.ap[0], [0, 2], qt.ap[1]])
                    kTh = qk[CC6 + h // 2][hb:hb + 64, :, :]
                else:
                    qTh = qk[h // 2][hb:hb + 64, :]
                    kTh = qk[CC6 + h // 2][hb:hb + 64, :]
                pts = []
                for mc in range(NC4):
                    pst = ps.tile([128, N], F32, tag="pst", name=f"pst{s}_{h}_{mc}", bufs=3)
                    ptile = sb.tile([128, N], BF16, tag="ptile", name=f"ptile{s}_{h}_{mc}", bufs=26)
                    if FP8_SCORES:
                        nc.tensor.matmul(pst[:], kTh[:, :, 128 * mc:128 * (mc + 1)],
                                         qTh, start=True, stop=True,
                                         perf_mode=mybir.MatmulPerfMode.DoubleRow)
                    else:
                        nc.tensor.matmul(pst[:], kTh[:, 128 * mc:128 * (mc + 1)], qTh,
                                         start=True, stop=True)
                    nc.scalar.activation(ptile[:], pst[:], EXP,
                                         scale=1.0 / (FP8_SCALE * FP8_SCALE)
                                         if FP8_SCORES else 1.0)
                    cnt["msk"] += 1
                    peng = (nc.vector, nc.gpsimd)[cnt["msk"] % 2]
                    with nc.allow_low_precision(reason="bf16 P"):
                        peng.tensor_mul(ptile[:], ptile[:], expm[mc][:])
                    pts.append(ptile)
                ptss[(s, h)] = pts

            def emit_pv(s, h):
                vsb = vsbs[s]
                g = h // 3          # head group (4 groups of 3)
                hg = h % 3
                pts = ptss.pop((s, h))
                if hg == 0:
                    for p in range(2):
                        pots[p] = ps.tile([128, 2 * 3 * (Dh + 1)], F32, tag="pot",
                                          name=f"pot{s}_{g}_{p}", bufs=2)
                for n4 in range(NC4):
                    pot = pots[n4 // 2][:, 195 * (n4 % 2):195 * (n4 % 2) + 195]
                    for mc in range(NC4):
                        nc.tensor.matmul(pot[:, 65 * hg:65 * (hg + 1)],
                                         pts[mc][:, 128 * n4:128 * (n4 + 1)],
                                         vsb[mc][:, 65 * h:65 * (h + 1)],
                                         start=(mc == 0), stop=(mc == NC4 - 1),
                                         skip_group_check=True)
                if hg == 2:
                    # drain the 3-head group: normalize by row-sums (col 64)
                    for n4 in range(NC4):
                        pot = pots[n4 // 2][:, 195 * (n4 % 2):195 * (n4 % 2) + 195]
                        potv = pot.rearrange("p (h e) -> p h e", e=Dh + 1)
                        lr = sb.tile([128, 3], F32, tag="lr", name=f"lr{s}_{g}_{n4}", bufs=4)
                        nc.vector.reciprocal(
                            lr[:].rearrange("p (h e) -> p h e", e=1),
                            potv[:, :, Dh:Dh + 1])
                        out = get(outs[s], n4, lambda n4=n4: sb.tile(
                            [128, C], BF16, tag="out", name=f"out_s{s}_{n4}", bufs=8))
                        b0, b1 = broadcast_tensor_aps(
                            potv[:, :, 0:Dh],
                            lr[:].rearrange("p (h e) -> p h e", e=1))
                        with nc.allow_low_precision(reason="bf16 out"):
                            nc.vector.tensor_mul(
                                out[:, 192 * g:192 * (g + 1)].rearrange(
                                    "p (h e) -> p h e", e=Dh), b0, b1)

            def emit_outT(s, n4, ccs=None):
                out = outs[s][n4]
                outT = get(outTs, s, lambda: sb.tile(
                    [128, CC6 * N], BF16, tag="outT", name=f"outT_s{s}", bufs=2))
                ccs = range(CC6) if ccs is None else ccs
                pt = ps.tile([128, 1024], BF16, tag="pst",
                             name=f"ot{s}_{n4}_{min(ccs)}", bufs=3)
                for cc in ccs:
                    ptc = pt[:, 128 * cc:128 * (cc + 1)]
                    nc.tensor.transpose(ptc, out[:, 128 * cc:128 * (cc + 1)], ident[:])
                    cnt["cp"] += 1
                    with nc.allow_low_precision(reason="bf16 outT"):
                        cp_eng2(cnt["cp"])(
                            outT[:, N * cc + 128 * n4:N * cc + 128 * (n4 + 1)], ptc)

            def emit_proj(s, n4):
                outT = outTs[s]
                if s == 1 and n4 == NC4 - 1:
                    # final unit: 3 narrow psum groups so the drain pipelines
                    osb = sb.tile([128, C], F32, tag="osb", name=f"osb{s}_{n4}", bufs=2)
                    bounds = [(0, 320), (320, 640), (640, 768)]
                    for third, (c0, c1) in enumerate(bounds):
                        w = c1 - c0
                        pr = ps.tile([128, 512], F32, tag="psq",
                                     name=f"pr{s}_{n4}_{third}", bufs=3)
                        for cc in range(CC6):
                            lhsT = outT[:, N * cc + 128 * n4:N * cc + 128 * (n4 + 1)]
                            nc.tensor.matmul(pr[:, 0:w], lhsT, projw[cc][:, c0:c1],
                                             start=(cc == 0), stop=(cc == CC6 - 1))
                        ceng = (nc.vector.tensor_copy, nc.scalar.copy,
                                nc.vector.tensor_copy)[third]
                        ceng(osb[:, c0:c1], pr[:, 0:w])
                        deng = (nc.sync, nc.scalar, nc.sync)[third]
                        deng.dma_start(y[s, 128 * n4:128 * (n4 + 1), c0:c1],
                                       osb[:, c0:c1])
                    return
                pra = ps.tile([128, 512], F32, tag="psq", name=f"pra{s}_{n4}", bufs=3)
                prb = ps.tile([128, 512], F32, tag="psq", name=f"prb{s}_{n4}", bufs=3)
                for cc in range(CC6):
                    lhsT = outT[:, N * cc + 128 * n4:N * cc + 128 * (n4 + 1)]
                    nc.tensor.matmul(pra[:], lhsT, projw[cc][:, 0:512],
                                     start=(cc == 0), stop=(cc == CC6 - 1))
                    nc.tensor.matmul(prb[:, 0:256], lhsT, projw[cc][:, 512:768],
                                     start=(cc == 0), stop=(cc == CC6 - 1))
                osb = sb.tile([128, C], F32, tag="osb", name=f"osb{s}_{n4}", bufs=2)
                nc.vector.tensor_copy(osb[:, 0:512], pra[:])
                d1, d2 = ((nc.sync, nc.scalar), (nc.scalar, nc.sync))[n4 % 2]
                d1.dma_start(y[s, 128 * n4:128 * (n4 + 1), 0:512], osb[:, 0:512])
                nc.scalar.copy(osb[:, 512:768], prb[:, 0:256])
                d2.dma_start(y[s, 128 * n4:128 * (n4 + 1), 512:768], osb[:, 512:768])

            # ---- interleaved schedule ----
            # scores run two heads ahead of PV; qk chunk pairs are emitted on
            # demand right before the scores that need them, so attention
            # starts early and qkv matmuls fill the exp-latency bubbles.

            def need_qk(s, h):
                j = h // 2
                if j not in qk_done[s]:
                    qk_done[s].add(j)
                    emit_qk(s, j)
                    emit_qk(s, CC6 + j)

            qk_done = [set(), set()]
            emit_weight_dmas()
            emit_transpose(0, 0, dve_only=True)
            emit_transpose(0, 1, dve_only=True)
            emit_v(0, 0)
            emit_transpose(0, 2, dve_only=True)
            emit_v(0, 1)
            emit_transpose(0, 3, dve_only=True)
            emit_mask_dmas()
            emit_v(0, 2)
            emit_v(0, 3)
            GRP_CCS0 = {2: [0], 5: [1, 2], 8: [3], 11: [4, 5]}
            # slice 0 attention interleaved with slice 1 early work
            e1 = [(emit_transpose, 1, 0), (emit_transpose, 1, 1),
                  (emit_v, 1, 0), (emit_transpose, 1, 2),
                  (emit_v, 1, 1), (emit_transpose, 1, 3),
                  (emit_v, 1, 2), (emit_v, 1, 3)] + \
                 [(emit_qk, 1, jc) for pair in
                  [(j, CC6 + j) for j in range(CC6)] for jc in pair]
            k = 0
            for hh in range(6):
                need_qk(0, hh)
                emit_scores(0, hh)
            for h in range(H):
                if h + 6 < H:
                    need_qk(0, h + 6)
                    emit_scores(0, h + 6)
                else:
                    # lookahead crosses into slice 1 so the exp stream
                    # never drains at the slice boundary
                    emit_scores(1, h + 6 - H)
                emit_pv(0, h)
                if h == 3:
                    emit_projw_dmas()
                tgt = (len(e1) * (h + 1)) // H
                while k < tgt:
                    f, a, b = e1[k]; f(a, b); k += 1
            # slice 1 attention; slice 0 outT+proj folded in
            p0 = [(emit_outT, 0, n4) for n4 in range(NC4)] + \
                 [(emit_proj, 0, n4) for n4 in range(NC4)]
            k = 0
            GRP_CCS = {2: [0], 5: [1, 2], 8: [3], 11: [4, 5]}
            for h in range(H):
                if h + 6 < H:
                    emit_scores(1, h + 6)
                emit_pv(1, h)
                if h in GRP_CCS and h != 11:
                    for n4 in range(NC4):
                        emit_outT(1, n4, GRP_CCS[h])
                tgt = (len(p0) * (h + 1)) // H
                while k < tgt:
                    u = p0[k]; u[0](*u[1:]); k += 1
            for n4 in range(NC4):
                emit_outT(1, n4, GRP_CCS[11])
                emit_proj(1, n4)

    nc.finalize()
    return nc


def _prep(x, mask, qkv_w, proj_w):
    scale = Dh ** -0.5
    qkv_wT = np.ascontiguousarray(qkv_w.T).astype(np.float32)
    qkv_wT[:, :C] *= scale
    maskT = np.ascontiguousarray(mask.reshape(N, N).T).astype(np.float32)
    if FP8_SCORES:
        # scores computed as (16 Wq x)(16 Wk x) = 256 S; exp scale undoes it
        qkv_wT[:, :2 * C] *= FP8_SCALE
    wqk = qkv_wT[:, :2 * C].astype(ml_dtypes.bfloat16)
    wv = np.ascontiguousarray(qkv_wT[:, 2 * C:]).astype(ml_dtypes.bfloat16)
    wp = np.ascontiguousarray(proj_w.T).astype(np.float32).astype(ml_dtypes.bfloat16)
    xbf = np.ascontiguousarray(x.reshape(B * T, N, C)).astype(ml_dtypes.bfloat16)
    return xbf, wqk, wv, wp, maskT


def make_sim_feed(inputs, core=0):
    """Feed dict for CoreSim replay of core `core` (test.py helper)."""
    x = np.asarray(inputs["x"])
    mask = np.asarray(inputs["mask"])
    qkv_w = np.asarray(inputs["qkv_w"]).astype(np.float32)
    proj_w = np.asarray(inputs["proj_w"]).astype(np.float32)
    xbf, wqk, wv, wp, maskT = _prep(np.asarray(x).astype(np.float32), mask, qkv_w, proj_w)
    return {"xs": xbf[SL * core:SL * (core + 1)], "wqk": wqk, "wv": wv,
            "wp": wp, "maskT": maskT}


def kernel(x, mask, qkv_w, q_bias, v_bias, proj_w, proj_b, _trace=False, _trace_kwargs=None):
    x, mask, qkv_w, proj_w = (np.asarray(a) for a in (x, mask, qkv_w, proj_w))
    q_bias, v_bias, proj_b = (np.asarray(a) for a in (q_bias, v_bias, proj_b))
    # biases folded in host-side only if nonzero (spec: all zeros). Assert to be safe.
    assert not np.any(q_bias) and not np.any(v_bias) and not np.any(proj_b), \
        "nonzero biases not supported by this kernel build"
    xbf, wqk, wv, wp, maskT = _prep(x.astype(np.float32), mask,
                                    qkv_w.astype(np.float32), proj_w.astype(np.float32))

    if "nc" not in _cache:
        _cache["nc"] = build_nc()
    nc = _cache["nc"]

    in_maps = []
    for c in range(NCORES):
        in_maps.append({
            "xs": xbf[SL * c:SL * (c + 1)],
            "wqk": wqk,
            "wv": wv,
            "wp": wp,
            "maskT": maskT,
        })
    res = run_bass_kernel_spmd(
        nc, in_maps, core_ids=list(range(NCORES)),
        trace=_trace, **(_trace_kwargs or {}),
    )
    out = np.concatenate([res.results[c]["y"] for c in range(NCORES)], axis=0)
    out = out.reshape(B, T, N, C)
    if _trace:
        return out, res
    return out
